# revision 57
# baseline (speedup 1.0000x reference)
"""Trainium2 Bass kernel for nn_BlockToChannelPool (segment softmax-pool).

Computation per batch sample b (B=32, N=4096, H=256, C=96):
  gate = tanh(x @ W1 + b1) @ W2 + b2                         (N,)
  per-channel softmax of gate over tokens with channel_ids==c, pad excluded
  w scaled by (1 + 0.1*emb[cancer_type])                     (C, N)
  pooled = w @ x                                             (C, H)
  tokens = ELU(LayerNorm(pooled @ Wp + bp))                  (C, H)

Sharding: pure data-parallel, B=32 split 4 samples/core across 8 cores.

Device algorithm (token math in bf16, f32 accumulation):
  - x is read twice from HBM in bf16: natural layout (pooling rhs, ACT DGE
    queue) and h-on-partition layout via the DMA xbar transposed read (gate
    matmul, SP DGE queue).
  - gate MLP on PE in "layout A": h1T (khid partitions, tokens free).
  - softmax without max subtraction (gate is bounded, exp cannot overflow);
    padded tokens get a zero row in the scaled one-hot so they drop out of
    both numerator and Z.
  - gate row (1, N) is bounced through a DRAM scratch (gpsimd SWDGE out) and
    re-read with the DMA transpose to land token-partitioned (128, 32);
    exp(gate+b2) on ACT after.
  - scaled one-hot lhsT (tokens, C) built per 128-token tile in one fused
    DVE tensor_scalar: (iota == ch) * s.
  - pooling matmul accumulates [x | 1] so column H is the softmax Z.
  - channel_proj + LayerNorm + ELU on-chip; rstd via exp(-0.5*ln(var+eps))
    so only two ACT table sets load per NEFF (exp/tanh set, ln/exp set).
  - all small constants ride in two packed blob DMAs.
  - has_ch (= Z > 0) and the batch-global any_ch masking happen on host.
"""

import numpy as np
import ml_dtypes

B, N, H, C = 32, 4096, 256, 96
NCORES = 8
SPC = B // NCORES            # samples per core
NT = N // 128                # token tiles per sample
LN_EPS = 1e-5

BF16 = ml_dtypes.bfloat16

# blob16 free-layout offsets (bf16)
O_W1, O_WP, O_IOTA, O_ID, O_W2, O_BP = 0, 256, 768, 896, 1024, 1025
F16 = 1282
# blob32 free-layout offsets (f32)
O_B1, O_B2, O_LNG, O_LNB, O_CTS = 0, 1, 2, 258, 514
F32T = 518

_CACHE: dict = {}


def _build_nc():
    import concourse.bacc as bacc
    import concourse.tile as tile
    from concourse.tile import add_dep_helper
    from concourse import mybir

    f32 = mybir.dt.float32
    bf16 = mybir.dt.bfloat16
    Alu = mybir.AluOpType
    Act = mybir.ActivationFunctionType

    nc = bacc.Bacc("TRN2", target_bir_lowering=False, num_swdge_queues=4)

    xb = nc.dram_tensor("xb", (SPC, N, H), bf16, kind="ExternalInput")
    chtp = nc.dram_tensor("chtp", (128, SPC, NT), f32, kind="ExternalInput")
    blob16 = nc.dram_tensor("blob16", (128, F16), bf16, kind="ExternalInput")
    blob32 = nc.dram_tensor("blob32", (128, F32T), f32, kind="ExternalInput")
    otok = nc.dram_tensor("otok", (SPC, C, H), f32, kind="ExternalOutput")
    import os
    DBG = os.environ.get("KDBG") == "1"
    if DBG:
        dbg_stp = nc.dram_tensor("dbg_stp", (128, NT), f32, kind="ExternalOutput")
        dbg_th = nc.dram_tensor("dbg_th", (128, N), bf16, kind="ExternalOutput")
        dbg_pooled = nc.dram_tensor("dbg_pooled", (C, H), bf16, kind="ExternalOutput")
        dbg_yps = nc.dram_tensor("dbg_yps", (C, H), f32, kind="ExternalOutput")
        dbg_srow = nc.dram_tensor("dbg_srow", (1, N), bf16, kind="ExternalOutput")
        dbg_stpbf = nc.dram_tensor("dbg_stpbf", (128, NT), bf16, kind="ExternalOutput")
    oz = nc.dram_tensor("oz", (C, SPC), f32, kind="ExternalOutput")

    with tile.TileContext(nc) as tc:
        with (
            tc.tile_pool(name="consts", bufs=1) as consts,
            tc.tile_pool(name="xt", bufs=1) as xt_pool,
            tc.tile_pool(name="xaug", bufs=1) as xaug_pool,
            tc.tile_pool(name="th", bufs=2) as th_pool,
            tc.tile_pool(name="srow", bufs=1) as srow_pool,
            tc.tile_pool(name="stp", bufs=SPC) as stp_pool,
            tc.tile_pool(name="oh", bufs=12) as oh_pool,
            tc.tile_pool(name="small", bufs=4) as small_pool,
            tc.tile_pool(name="outp", bufs=2) as out_pool,
            tc.tile_pool(name="ps_h1", bufs=2, space="PSUM") as ps_h1,
            tc.tile_pool(name="ps_g", bufs=2, space="PSUM") as ps_g,
            tc.tile_pool(name="ps_pool", bufs=2, space="PSUM") as ps_pool,
            tc.tile_pool(name="ps_yt", bufs=1, space="PSUM") as ps_yt,
            tc.tile_pool(name="dram", bufs=1, space="DRAM") as dram_pool,
        ):
            # ---- packed constants + all channel ids ----
            c16 = consts.tile([128, F16], bf16)
            nc.scalar.dma_start(out=c16, in_=blob16[:])
            c32 = consts.tile([128, F32T], f32)
            nc.scalar.dma_start(out=c32, in_=blob32[:])
            ch_all = consts.tile([128, SPC, NT], f32)
            nc.scalar.dma_start(out=ch_all, in_=chtp[:])

            w1_sb = c16[:, O_W1:O_W1 + 256].rearrange("p (c k) -> p c k", c=2)
            wp_sb = c16[:, O_WP:O_WP + 512].rearrange("p (c k) -> p c k", c=2)
            iota_sb = c16[:, O_IOTA:O_IOTA + 128]
            id_sb = c16[:, O_ID:O_ID + 128]
            w2_sb = c16[:, O_W2:O_W2 + 1]
            bp_sb = c16[0:1, O_BP:O_BP + H]
            b1_sb = c32[:, O_B1:O_B1 + 1]
            b2_sb = c32[:, O_B2:O_B2 + 1]
            lng_sb = c32[0:C, O_LNG:O_LNG + H]
            lnb_sb = c32[0:C, O_LNB:O_LNB + H]
            cts_sb = c32[0:C, O_CTS:O_CTS + SPC]

            ones96 = consts.tile([1, C], bf16)
            nc.vector.memset(ones96, 1.0)

            # cross-phase stashes
            z_all = consts.tile([C, SPC], f32)



            # ---- big loads, all up front: xT on SP queue, x_aug on ACT ----
            # One xbar-transposed read per sample: x[s] viewed as (N*2, 128)
            # transposes to (128, N, 2) with the two h-chunks interleaved in
            # the last axis; matmul reads them back with stride-2 APs.
            xtss = []
            for s in range(SPC):
                xt = xt_pool.tile([128, N, 2], bf16, tag=f"xt{s}")
                nc.sync.dma_start(
                    out=xt,
                    in_=xb[s].rearrange("n (c p) -> (n c) p", p=128),
                    transpose=True,
                )
                xtss.append([xt[:, :, 0], xt[:, :, 1]])
            x_augs = [None] * SPC
            for s in [0, 1, 3, 2]:
                x_aug = xaug_pool.tile([128, NT, 260], bf16, tag=f"xaug{s}")
                nc.vector.memset(x_aug[:, :, H:H + 1], 1.0)
                xin = xb[s].rearrange("(t p) h -> p t h", p=128)
                eng = nc.gpsimd if s < 2 else nc.sync
                eng.dma_start(out=x_aug[:, :, 0:H], in_=xin)
                x_augs[s] = x_aug

            # ================= PHASE 1: gate MLP per sample =================
            stps = []
            for s in range(SPC):
                xts = xtss[s]
                th = th_pool.tile([128, N], bf16, tag="th")
                for fc in range(8):
                    h1_ps = ps_h1.tile([128, 512], f32, tag="h1")
                    j0 = fc * 512
                    for hc in range(2):
                        nc.tensor.matmul(
                            h1_ps,
                            lhsT=w1_sb[:, hc, :],
                            rhs=xts[hc][:, j0:j0 + 512],
                            start=(hc == 0),
                            stop=(hc == 1),
                        )
                    nc.scalar.activation(
                        out=th[:, fc * 512:(fc + 1) * 512],
                        in_=h1_ps, func=Act.Tanh, bias=b1_sb, scale=1.0,
                    )

                s_row = srow_pool.tile([1, N], bf16, tag=f"srow{s}")
                for fc in range(8):
                    g_ps = ps_g.tile([1, 512], f32, tag="g")
                    nc.tensor.matmul(
                        g_ps, lhsT=w2_sb, rhs=th[:, fc * 512:(fc + 1) * 512],
                        start=True, stop=True,
                    )
                    dst = s_row[:, fc * 512:(fc + 1) * 512]
                    nc.vector.tensor_copy(out=dst, in_=g_ps)

                if DBG and s == 0:
                    nc.sync.dma_start(out=dbg_srow[:], in_=s_row)
                # bounce via DRAM to token-partition the gate row
                srow_d = dram_pool.tile([NT, 128], bf16, tag=f"srd{s}")
                # bounce DMAs must not share an engine with any engine that
                # wrote s_row (same-engine write -> DMA read misses the drain
                # sem), so all g-copies above are DVE and the bounce rides the
                # ACT DGE queue.
                nc.scalar.dma_start(out=srow_d, in_=s_row)
                stp_bf = srow_pool.tile([128, NT], bf16, tag=f"stpb{s}")
                nc.scalar.dma_start(out=stp_bf, in_=srow_d[:], transpose=True)
                if DBG and s == 0:
                    nc.sync.dma_start(out=dbg_stpbf[:], in_=stp_bf)
                stp = stp_pool.tile([128, NT], f32, tag="stp")
                nc.scalar.activation(
                    out=stp, in_=stp_bf, func=Act.Exp, bias=b2_sb, scale=1.0
                )
                stps.append(stp)
                if DBG and s == 0:
                    nc.sync.dma_start(out=dbg_stp[:], in_=stp)
                    nc.sync.dma_start(out=dbg_th[:], in_=th)

            # ========== PHASE 2: pooling + proj + LN + ELU per sample ==========
            for s in range(SPC):
                x_aug, stp = x_augs[s], stps[s]
                pool_ps = ps_pool.tile([128, H + 1], f32, tag="pool")
                for t in range(NT):
                    oh = oh_pool.tile([128, 128], bf16, tag="oh")
                    eng_oh = nc.vector if t % 8 != 7 else nc.gpsimd
                    eng_oh.tensor_scalar(
                        out=oh, in0=iota_sb,
                        scalar1=ch_all[:, s, t:t + 1],
                        scalar2=stp[:, t:t + 1],
                        op0=Alu.is_equal, op1=Alu.mult,
                    )
                    nc.tensor.matmul(
                        pool_ps, lhsT=oh, rhs=x_aug[:, t, 0:H + 1],
                        start=(t == 0), stop=(t == NT - 1),
                    )

                nc.vector.tensor_copy(out=z_all[:, s:s + 1], in_=pool_ps[0:C, H:H + 1])
                zmax = small_pool.tile([C, 1], f32, tag="zmax")
                nc.vector.tensor_scalar_max(
                    out=zmax, in0=pool_ps[0:C, H:H + 1], scalar1=1e-30
                )
                rec = small_pool.tile([C, 1], f32, tag="rec")
                nc.vector.reciprocal(out=rec, in_=zmax)
                scl = small_pool.tile([C, 1], f32, tag="scl")
                nc.vector.tensor_tensor(
                    out=scl, in0=rec, in1=cts_sb[:, s:s + 1], op=Alu.mult,
                )
                pooled_sb = small_pool.tile([C, H], bf16, tag="pooled")
                nc.vector.tensor_scalar_mul(
                    out=pooled_sb, in0=pool_ps[0:C, 0:H], scalar1=scl
                )
                if DBG and s == 0:
                    nc.sync.dma_start(out=dbg_pooled[:], in_=pooled_sb)
                poolT = small_pool.tile([128, 2, C], bf16, tag="poolT")
                for hc in range(2):
                    tp_ps = ps_yt.tile([128, C], bf16, tag="yt")
                    nc.tensor.transpose(
                        tp_ps, pooled_sb[:, hc * 128:(hc + 1) * 128], id_sb[0:C, 0:C]
                    )
                    nc.scalar.copy(out=poolT[:, hc, :], in_=tp_ps)
                y_ps = ps_yt.tile([C, H], f32, tag="yt")
                for hc in range(2):
                    nc.tensor.matmul(
                        y_ps, lhsT=poolT[:, hc, :], rhs=wp_sb[:, hc, :],
                        start=(hc == 0), stop=False,
                    )
                nc.tensor.matmul(y_ps, lhsT=ones96, rhs=bp_sb, start=False, stop=True)
                # LayerNorm: rstd = exp(-0.5*ln(var+eps)), per sample
                if DBG and s == 0:
                    yf = small_pool.tile([C, H], f32, tag="dbgyf")
                    nc.vector.tensor_copy(out=yf, in_=y_ps)
                    nc.sync.dma_start(out=dbg_yps[:], in_=yf)
                st6 = small_pool.tile([C, 6], f32, tag="st6")
                nc.vector.bn_stats(out=st6, in_=y_ps)
                mv = small_pool.tile([C, 2], f32, tag="mv")
                nc.vector.bn_aggr(out=mv, in_=st6)
                ve = small_pool.tile([C, 1], f32, tag="ve")
                nc.vector.tensor_scalar_add(out=ve, in0=mv[:, 1:2], scalar1=LN_EPS)
                # rstd = 1/sqrt(ve) via Quake seed + 2 Newton steps, all DVE
                # (keeps the whole kernel inside one ACT table set).
                i32 = mybir.dt.int32
                ti = small_pool.tile([C, 1], i32, tag="ti")
                nc.vector.tensor_scalar(
                    out=ti, in0=ve.bitcast(i32), scalar1=1, scalar2=None,
                    op0=Alu.arith_shift_right,
                )
                si = small_pool.tile([C, 1], i32, tag="si")
                nc.vector.tensor_scalar(
                    out=si, in0=ti, scalar1=-1.0, scalar2=float(0x5F3759DF),
                    op0=Alu.mult, op1=Alu.add,
                )
                vh = small_pool.tile([C, 1], f32, tag="vh")
                nc.vector.tensor_scalar_mul(out=vh, in0=ve, scalar1=-0.5)
                rstd = si.bitcast(f32)
                for _ in range(1):
                    qa = small_pool.tile([C, 1], f32, tag="qa")
                    nc.vector.tensor_tensor(out=qa, in0=rstd, in1=rstd, op=Alu.mult)
                    qc = small_pool.tile([C, 1], f32, tag="qc")
                    nc.vector.tensor_scalar(
                        out=qc, in0=qa, scalar1=vh, scalar2=1.5,
                        op0=Alu.mult, op1=Alu.add,
                    )
                    rn = small_pool.tile([C, 1], f32, tag="rn")
                    nc.vector.tensor_tensor(out=rn, in0=rstd, in1=qc, op=Alu.mult)
                    rstd = rn
                yn = out_pool.tile([C, H], f32, tag="yn")
                nc.vector.tensor_scalar(
                    out=yn, in0=y_ps, scalar1=mv[:, 0:1], scalar2=rstd,
                    op0=Alu.subtract, op1=Alu.mult,
                )
                nc.vector.tensor_tensor(out=yn, in0=yn, in1=lng_sb, op=Alu.mult)
                nc.vector.tensor_tensor(out=yn, in0=yn, in1=lnb_sb, op=Alu.add)
                # ELU = max(v,0) + exp(min(v,0)) - 1
                mneg = out_pool.tile([C, H], f32, tag="mneg")
                nc.gpsimd.tensor_scalar_min(out=mneg, in0=yn, scalar1=0.0)
                ee = out_pool.tile([C, H], f32, tag="ee")
                nc.scalar.activation(out=ee, in_=mneg, func=Act.Exp)
                mpos = out_pool.tile([C, H], f32, tag="mpos")
                nc.gpsimd.tensor_scalar_max(out=mpos, in0=yn, scalar1=0.0)
                res = out_pool.tile([C, H], f32, tag="res")
                nc.vector.scalar_tensor_tensor(
                    out=res, in0=ee, scalar=-1.0, in1=mpos,
                    op0=Alu.add, op1=Alu.add,
                )
                nc.gpsimd.dma_start(out=otok[s], in_=res)
            nc.gpsimd.dma_start(out=oz[:], in_=z_all)

    nc.compile()
    return nc


def _get_nc():
    if "nc" not in _CACHE:
        _CACHE["nc"] = _build_nc()
    return _CACHE["nc"]


def _marshal(inputs):
    x = np.ascontiguousarray(np.asarray(inputs["x"], dtype=np.float32))
    cancer_type = np.asarray(inputs["cancer_type"]).astype(np.int64)
    channel_ids = np.asarray(inputs["channel_ids"]).astype(np.int64)
    pad_mask = np.asarray(inputs["pad_mask"]).astype(bool)
    W1 = np.asarray(inputs["W1"], dtype=np.float32)
    b1 = np.asarray(inputs["b1"], dtype=np.float32)
    W2 = np.asarray(inputs["W2"], dtype=np.float32)
    b2 = np.asarray(inputs["b2"], dtype=np.float32)
    emb = np.asarray(inputs["emb"], dtype=np.float32)
    Wp = np.asarray(inputs["Wp"], dtype=np.float32)
    bp = np.asarray(inputs["bp"], dtype=np.float32)
    ln_g = np.asarray(inputs["ln_g"], dtype=np.float32)
    ln_b = np.asarray(inputs["ln_b"], dtype=np.float32)

    xb = x.astype(BF16)
    ch_f = np.where(pad_mask, np.float32(-1.0), channel_ids.astype(np.float32))
    # token-partition layout per sample: [p, t] with n = t*128 + p
    chtp = ch_f.reshape(B, NT, 128).transpose(2, 0, 1)      # (128, B, NT)
    ctscale = (1.0 + 0.1 * emb[cancer_type]).astype(np.float32)  # (B, C)

    blob16 = np.zeros((128, F16), dtype=BF16)
    blob16[:, O_W1:O_W1 + 256] = (
        W1.astype(BF16).reshape(2, 128, 128).transpose(1, 0, 2).reshape(128, 256)
    )
    blob16[:, O_WP:O_WP + 512] = (
        Wp.astype(BF16).reshape(2, 128, 256).transpose(1, 0, 2).reshape(128, 512)
    )
    blob16[:, O_IOTA:O_IOTA + 128] = np.arange(128, dtype=np.float32).astype(BF16)[None, :]
    blob16[:, O_ID:O_ID + 128] = np.eye(128, dtype=np.float32).astype(BF16)
    blob16[:, O_W2] = W2.astype(BF16)
    blob16[0, O_BP:O_BP + H] = bp.astype(BF16)

    blob32_base = np.zeros((128, F32T), dtype=np.float32)
    blob32_base[:, O_B1] = b1
    blob32_base[:, O_B2] = np.float32(b2)
    blob32_base[:, O_LNG:O_LNG + H] = ln_g[None, :]
    blob32_base[:, O_LNB:O_LNB + H] = ln_b[None, :]

    in_maps = []
    for c in range(NCORES):
        sl = slice(c * SPC, (c + 1) * SPC)
        blob32 = blob32_base.copy()
        blob32[0:C, O_CTS:O_CTS + SPC] = ctscale[sl].T
        in_maps.append({
            "xb": np.ascontiguousarray(xb[sl]),
            "chtp": np.ascontiguousarray(chtp[:, sl, :]),
            "blob16": blob16,
            "blob32": blob32,
        })
    return in_maps


def kernel(**inputs) -> tuple:
    in_maps = _marshal(inputs)
    from concourse import bass_utils
    nc = _get_nc()
    _CACHE["in_maps"] = in_maps
    res = bass_utils.run_bass_kernel_spmd(nc, in_maps, core_ids=list(range(NCORES)))

    tokens = np.empty((B, C, H), dtype=np.float32)
    Z = np.empty((B, C), dtype=np.float32)
    for c in range(NCORES):
        out = res.results[c]
        tokens[c * SPC:(c + 1) * SPC] = out["otok"]
        Z[c * SPC:(c + 1) * SPC] = out["oz"].T
    has_ch = Z > 0
    any_ch = has_ch.any(axis=0)
    tokens = np.where(any_ch[None, :, None], tokens, np.float32(0.0))
    return tokens, has_ch


# revision 65
# speedup vs baseline: 1.0062x; 1.0062x over previous
"""Trainium2 Bass kernel for nn_BlockToChannelPool (segment softmax-pool).

Computation per batch sample b (B=32, N=4096, H=256, C=96):
  gate = tanh(x @ W1 + b1) @ W2 + b2                         (N,)
  per-channel softmax of gate over tokens with channel_ids==c, pad excluded
  w scaled by (1 + 0.1*emb[cancer_type])                     (C, N)
  pooled = w @ x                                             (C, H)
  tokens = ELU(LayerNorm(pooled @ Wp + bp))                  (C, H)

Sharding: pure data-parallel, B=32 split 4 samples/core across 8 cores.

Device algorithm (token math in bf16, f32 accumulation):
  - x is read twice from HBM in bf16: natural layout (pooling rhs, ACT DGE
    queue) and h-on-partition layout via the DMA xbar transposed read (gate
    matmul, SP DGE queue).
  - gate MLP on PE in "layout A": h1T (khid partitions, tokens free).
  - softmax without max subtraction (gate is bounded, exp cannot overflow);
    padded tokens get a zero row in the scaled one-hot so they drop out of
    both numerator and Z.
  - gate row (1, N) is bounced through a DRAM scratch (gpsimd SWDGE out) and
    re-read with the DMA transpose to land token-partitioned (128, 32);
    exp(gate+b2) on ACT after.
  - scaled one-hot lhsT (tokens, C) built per 128-token tile in one fused
    DVE tensor_scalar: (iota == ch) * s.
  - pooling matmul accumulates [x | 1] so column H is the softmax Z.
  - channel_proj + LayerNorm + ELU on-chip; rstd via exp(-0.5*ln(var+eps))
    so only two ACT table sets load per NEFF (exp/tanh set, ln/exp set).
  - all small constants ride in two packed blob DMAs.
  - has_ch (= Z > 0) and the batch-global any_ch masking happen on host.
"""

import numpy as np
import ml_dtypes

B, N, H, C = 32, 4096, 256, 96
NCORES = 8
SPC = B // NCORES            # samples per core
NT = N // 128                # token tiles per sample
LN_EPS = 1e-5

BF16 = ml_dtypes.bfloat16

# blob16 free-layout offsets (bf16)
O_W1, O_WP, O_IOTA, O_ID, O_W2, O_BP = 0, 256, 768, 896, 1024, 1025
F16 = 1282
# blob32 free-layout offsets (f32)
O_B1, O_B2, O_LNG, O_LNB, O_CTS = 0, 1, 2, 258, 514
F32T = 518

_CACHE: dict = {}


def _build_nc():
    import concourse.bacc as bacc
    import concourse.tile as tile
    from concourse.tile import add_dep_helper
    from concourse import mybir

    f32 = mybir.dt.float32
    bf16 = mybir.dt.bfloat16
    Alu = mybir.AluOpType
    Act = mybir.ActivationFunctionType

    nc = bacc.Bacc("TRN2", target_bir_lowering=False, num_swdge_queues=4)

    xb = nc.dram_tensor("xb", (SPC, N, H), bf16, kind="ExternalInput")
    chtp = nc.dram_tensor("chtp", (128, SPC, NT), f32, kind="ExternalInput")
    blob16 = nc.dram_tensor("blob16", (128, F16), bf16, kind="ExternalInput")
    blob32 = nc.dram_tensor("blob32", (128, F32T), f32, kind="ExternalInput")
    otok = nc.dram_tensor("otok", (SPC, C, H), f32, kind="ExternalOutput")
    import os
    DBG = os.environ.get("KDBG") == "1"
    if DBG:
        dbg_stp = nc.dram_tensor("dbg_stp", (128, NT), f32, kind="ExternalOutput")
        dbg_th = nc.dram_tensor("dbg_th", (128, N), bf16, kind="ExternalOutput")
        dbg_pooled = nc.dram_tensor("dbg_pooled", (C, H), bf16, kind="ExternalOutput")
        dbg_yps = nc.dram_tensor("dbg_yps", (C, H), f32, kind="ExternalOutput")
        dbg_srow = nc.dram_tensor("dbg_srow", (1, N), bf16, kind="ExternalOutput")
        dbg_stpbf = nc.dram_tensor("dbg_stpbf", (128, NT), bf16, kind="ExternalOutput")
    oz = nc.dram_tensor("oz", (C, SPC), f32, kind="ExternalOutput")

    with tile.TileContext(nc) as tc:
        with (
            tc.tile_pool(name="consts", bufs=1) as consts,
            tc.tile_pool(name="xt", bufs=1) as xt_pool,
            tc.tile_pool(name="xaug", bufs=1) as xaug_pool,
            tc.tile_pool(name="th", bufs=2) as th_pool,
            tc.tile_pool(name="srow", bufs=1) as srow_pool,
            tc.tile_pool(name="stp", bufs=SPC) as stp_pool,
            tc.tile_pool(name="oh", bufs=12) as oh_pool,
            tc.tile_pool(name="small", bufs=4) as small_pool,
            tc.tile_pool(name="outp", bufs=2) as out_pool,
            tc.tile_pool(name="ps_h1", bufs=2, space="PSUM") as ps_h1,
            tc.tile_pool(name="ps_g", bufs=2, space="PSUM") as ps_g,
            tc.tile_pool(name="ps_pool", bufs=2, space="PSUM") as ps_pool,
            tc.tile_pool(name="ps_yt", bufs=1, space="PSUM") as ps_yt,
            tc.tile_pool(name="dram", bufs=1, space="DRAM") as dram_pool,
        ):
            # ---- packed constants + all channel ids ----
            c16 = consts.tile([128, F16], bf16)
            nc.scalar.dma_start(out=c16, in_=blob16[:])
            c32 = consts.tile([128, F32T], f32)
            nc.scalar.dma_start(out=c32, in_=blob32[:])
            ch_all = consts.tile([128, SPC, NT], f32)
            nc.scalar.dma_start(out=ch_all, in_=chtp[:])

            w1_sb = c16[:, O_W1:O_W1 + 256].rearrange("p (c k) -> p c k", c=2)
            wp_sb = c16[:, O_WP:O_WP + 512].rearrange("p (c k) -> p c k", c=2)
            iota_sb = c16[:, O_IOTA:O_IOTA + 128]
            id_sb = c16[:, O_ID:O_ID + 128]
            w2_sb = c16[:, O_W2:O_W2 + 1]
            bp_sb = c16[0:1, O_BP:O_BP + H]
            b1_sb = c32[:, O_B1:O_B1 + 1]
            b2_sb = c32[:, O_B2:O_B2 + 1]
            lng_sb = c32[0:C, O_LNG:O_LNG + H]
            lnb_sb = c32[0:C, O_LNB:O_LNB + H]
            cts_sb = c32[0:C, O_CTS:O_CTS + SPC]

            ones96 = consts.tile([1, C], bf16)
            nc.vector.memset(ones96, 1.0)

            # cross-phase stashes
            z_all = consts.tile([C, SPC], f32)



            # ---- big loads, all up front: xT on SP queue, x_aug on ACT ----
            # One xbar-transposed read per sample: x[s] viewed as (N*2, 128)
            # transposes to (128, N, 2) with the two h-chunks interleaved in
            # the last axis; matmul reads them back with stride-2 APs.
            xtss = []
            for s in range(SPC):
                xt = xt_pool.tile([128, N, 2], bf16, tag=f"xt{s}")
                nc.sync.dma_start(
                    out=xt,
                    in_=xb[s].rearrange("n (c p) -> (n c) p", p=128),
                    transpose=True,
                )
                xtss.append([xt[:, :, 0], xt[:, :, 1]])
            x_augs = [None] * SPC
            for s in [0, 1, 3, 2]:
                x_aug = xaug_pool.tile([128, NT, 260], bf16, tag=f"xaug{s}")
                nc.vector.memset(x_aug[:, :, H:H + 1], 1.0)
                xin = xb[s].rearrange("(t p) h -> p t h", p=128)
                eng = nc.gpsimd if s < 2 else nc.sync
                eng.dma_start(out=x_aug[:, :, 0:H], in_=xin)
                x_augs[s] = x_aug

            # ================= PHASE 1: gate MLP per sample =================
            stps = []
            for s in range(SPC):
                xts = xtss[s]
                th = th_pool.tile([128, N], bf16, tag="th")
                for fc in range(8):
                    h1_ps = ps_h1.tile([128, 512], f32, tag="h1")
                    j0 = fc * 512
                    for hc in range(2):
                        nc.tensor.matmul(
                            h1_ps,
                            lhsT=w1_sb[:, hc, :],
                            rhs=xts[hc][:, j0:j0 + 512],
                            start=(hc == 0),
                            stop=(hc == 1),
                        )
                    nc.scalar.activation(
                        out=th[:, fc * 512:(fc + 1) * 512],
                        in_=h1_ps, func=Act.Tanh, bias=b1_sb, scale=1.0,
                    )

                s_row = srow_pool.tile([1, N], bf16, tag=f"srow{s}")
                for fc in range(8):
                    g_ps = ps_g.tile([1, 512], f32, tag="g")
                    nc.tensor.matmul(
                        g_ps, lhsT=w2_sb, rhs=th[:, fc * 512:(fc + 1) * 512],
                        start=True, stop=True,
                    )
                    dst = s_row[:, fc * 512:(fc + 1) * 512]
                    nc.vector.tensor_copy(out=dst, in_=g_ps)

                if DBG and s == 0:
                    nc.sync.dma_start(out=dbg_srow[:], in_=s_row)
                # bounce via DRAM to token-partition the gate row
                srow_d = dram_pool.tile([NT, 128], bf16, tag=f"srd{s}")
                # bounce DMAs must not share an engine with any engine that
                # wrote s_row (same-engine write -> DMA read misses the drain
                # sem), so all g-copies above are DVE and the bounce rides the
                # ACT DGE queue.
                nc.scalar.dma_start(out=srow_d, in_=s_row)
                stp_bf = srow_pool.tile([128, NT], bf16, tag=f"stpb{s}")
                nc.scalar.dma_start(out=stp_bf, in_=srow_d[:], transpose=True)
                if DBG and s == 0:
                    nc.sync.dma_start(out=dbg_stpbf[:], in_=stp_bf)
                stp = stp_pool.tile([128, NT], f32, tag="stp")
                nc.scalar.activation(
                    out=stp, in_=stp_bf, func=Act.Exp, bias=b2_sb, scale=1.0
                )
                stps.append(stp)
                if DBG and s == 0:
                    nc.sync.dma_start(out=dbg_stp[:], in_=stp)
                    nc.sync.dma_start(out=dbg_th[:], in_=th)

            # ========== PHASE 2: pooling + proj + LN + ELU per sample ==========
            for s in range(SPC):
                x_aug, stp = x_augs[s], stps[s]
                pool_ps = ps_pool.tile([128, H + 1], f32, tag="pool")
                for t in range(NT):
                    oh = oh_pool.tile([128, 128], bf16, tag="oh")
                    eng_oh = nc.vector if t % 8 != 7 else nc.gpsimd
                    eng_oh.tensor_scalar(
                        out=oh, in0=iota_sb,
                        scalar1=ch_all[:, s, t:t + 1],
                        scalar2=stp[:, t:t + 1],
                        op0=Alu.is_equal, op1=Alu.mult,
                    )
                    nc.tensor.matmul(
                        pool_ps, lhsT=oh, rhs=x_aug[:, t, 0:H + 1],
                        start=(t == 0), stop=(t == NT - 1),
                    )

                nc.vector.tensor_copy(out=z_all[:, s:s + 1], in_=pool_ps[0:C, H:H + 1])
                zmax = small_pool.tile([C, 1], f32, tag="zmax")
                nc.vector.tensor_scalar_max(
                    out=zmax, in0=pool_ps[0:C, H:H + 1], scalar1=1e-30
                )
                rec = small_pool.tile([C, 1], f32, tag="rec")
                nc.vector.reciprocal(out=rec, in_=zmax)
                pooled_sb = small_pool.tile([C, H], bf16, tag="pooled")
                nc.vector.tensor_scalar(
                    out=pooled_sb, in0=pool_ps[0:C, 0:H],
                    scalar1=rec, scalar2=cts_sb[:, s:s + 1],
                    op0=Alu.mult, op1=Alu.mult,
                )
                if DBG and s == 0:
                    nc.sync.dma_start(out=dbg_pooled[:], in_=pooled_sb)
                poolT = small_pool.tile([128, 2, C], bf16, tag="poolT")
                for hc in range(2):
                    tp_ps = ps_yt.tile([128, C], bf16, tag="yt")
                    nc.tensor.transpose(
                        tp_ps, pooled_sb[:, hc * 128:(hc + 1) * 128], id_sb[0:C, 0:C]
                    )
                    nc.scalar.copy(out=poolT[:, hc, :], in_=tp_ps)
                y_ps = ps_yt.tile([C, H], f32, tag="yt")
                for hc in range(2):
                    nc.tensor.matmul(
                        y_ps, lhsT=poolT[:, hc, :], rhs=wp_sb[:, hc, :],
                        start=(hc == 0), stop=False,
                    )
                nc.tensor.matmul(y_ps, lhsT=ones96, rhs=bp_sb, start=False, stop=True)
                # LayerNorm: rstd = exp(-0.5*ln(var+eps)), per sample
                if DBG and s == 0:
                    yf = small_pool.tile([C, H], f32, tag="dbgyf")
                    nc.vector.tensor_copy(out=yf, in_=y_ps)
                    nc.sync.dma_start(out=dbg_yps[:], in_=yf)
                st6 = small_pool.tile([C, 6], f32, tag="st6")
                nc.vector.bn_stats(out=st6, in_=y_ps)
                mv = small_pool.tile([C, 2], f32, tag="mv")
                nc.vector.bn_aggr(out=mv, in_=st6)
                ve = small_pool.tile([C, 1], f32, tag="ve")
                nc.vector.tensor_scalar_add(out=ve, in0=mv[:, 1:2], scalar1=LN_EPS)
                # rstd = 1/sqrt(ve) via Quake seed + 2 Newton steps, all DVE
                # (keeps the whole kernel inside one ACT table set).
                i32 = mybir.dt.int32
                ti = small_pool.tile([C, 1], i32, tag="ti")
                nc.vector.tensor_scalar(
                    out=ti, in0=ve.bitcast(i32), scalar1=1, scalar2=None,
                    op0=Alu.arith_shift_right,
                )
                si = small_pool.tile([C, 1], i32, tag="si")
                nc.vector.tensor_scalar(
                    out=si, in0=ti, scalar1=-1.0, scalar2=float(0x5F3759DF),
                    op0=Alu.mult, op1=Alu.add,
                )
                vh = small_pool.tile([C, 1], f32, tag="vh")
                nc.vector.tensor_scalar_mul(out=vh, in0=ve, scalar1=-0.5)
                rstd = si.bitcast(f32)
                for _ in range(1):
                    qa = small_pool.tile([C, 1], f32, tag="qa")
                    nc.vector.tensor_tensor(out=qa, in0=rstd, in1=rstd, op=Alu.mult)
                    qc = small_pool.tile([C, 1], f32, tag="qc")
                    nc.vector.tensor_scalar(
                        out=qc, in0=qa, scalar1=vh, scalar2=1.5,
                        op0=Alu.mult, op1=Alu.add,
                    )
                    rn = small_pool.tile([C, 1], f32, tag="rn")
                    nc.vector.tensor_tensor(out=rn, in0=rstd, in1=qc, op=Alu.mult)
                    rstd = rn
                t1 = out_pool.tile([C, H], f32, tag="t1")
                nc.vector.tensor_scalar(
                    out=t1, in0=y_ps, scalar1=mv[:, 0:1], scalar2=None,
                    op0=Alu.subtract,
                )
                nc.vector.tensor_tensor(out=t1, in0=t1, in1=lng_sb, op=Alu.mult)
                yn = out_pool.tile([C, H], f32, tag="yn")
                nc.vector.tensor_scalar_mul(out=yn, in0=t1, scalar1=rstd)
                nc.vector.tensor_tensor(out=yn, in0=yn, in1=lnb_sb, op=Alu.add)
                # ELU = max(v,0) + exp(min(v,0)) - 1
                mneg = out_pool.tile([C, H], f32, tag="mneg")
                nc.vector.tensor_scalar_min(out=mneg, in0=yn, scalar1=0.0)
                ee = out_pool.tile([C, H], f32, tag="ee")
                nc.scalar.activation(out=ee, in_=mneg, func=Act.Exp)
                mpos = out_pool.tile([C, H], f32, tag="mpos")
                nc.vector.tensor_scalar_max(out=mpos, in0=yn, scalar1=0.0)
                res = out_pool.tile([C, H], f32, tag="res")
                nc.vector.scalar_tensor_tensor(
                    out=res, in0=ee, scalar=-1.0, in1=mpos,
                    op0=Alu.add, op1=Alu.add,
                )
                nc.scalar.dma_start(out=otok[s], in_=res)
            nc.scalar.dma_start(out=oz[:], in_=z_all)

    nc.compile()
    return nc


def _get_nc():
    if "nc" not in _CACHE:
        _CACHE["nc"] = _build_nc()
    return _CACHE["nc"]


def _marshal(inputs):
    x = np.ascontiguousarray(np.asarray(inputs["x"], dtype=np.float32))
    cancer_type = np.asarray(inputs["cancer_type"]).astype(np.int64)
    channel_ids = np.asarray(inputs["channel_ids"]).astype(np.int64)
    pad_mask = np.asarray(inputs["pad_mask"]).astype(bool)
    W1 = np.asarray(inputs["W1"], dtype=np.float32)
    b1 = np.asarray(inputs["b1"], dtype=np.float32)
    W2 = np.asarray(inputs["W2"], dtype=np.float32)
    b2 = np.asarray(inputs["b2"], dtype=np.float32)
    emb = np.asarray(inputs["emb"], dtype=np.float32)
    Wp = np.asarray(inputs["Wp"], dtype=np.float32)
    bp = np.asarray(inputs["bp"], dtype=np.float32)
    ln_g = np.asarray(inputs["ln_g"], dtype=np.float32)
    ln_b = np.asarray(inputs["ln_b"], dtype=np.float32)

    xb = x.astype(BF16)
    ch_f = np.where(pad_mask, np.float32(-1.0), channel_ids.astype(np.float32))
    # token-partition layout per sample: [p, t] with n = t*128 + p
    chtp = ch_f.reshape(B, NT, 128).transpose(2, 0, 1)      # (128, B, NT)
    ctscale = (1.0 + 0.1 * emb[cancer_type]).astype(np.float32)  # (B, C)

    blob16 = np.zeros((128, F16), dtype=BF16)
    blob16[:, O_W1:O_W1 + 256] = (
        W1.astype(BF16).reshape(2, 128, 128).transpose(1, 0, 2).reshape(128, 256)
    )
    blob16[:, O_WP:O_WP + 512] = (
        Wp.astype(BF16).reshape(2, 128, 256).transpose(1, 0, 2).reshape(128, 512)
    )
    blob16[:, O_IOTA:O_IOTA + 128] = np.arange(128, dtype=np.float32).astype(BF16)[None, :]
    blob16[:, O_ID:O_ID + 128] = np.eye(128, dtype=np.float32).astype(BF16)
    blob16[:, O_W2] = W2.astype(BF16)
    blob16[0, O_BP:O_BP + H] = bp.astype(BF16)

    blob32_base = np.zeros((128, F32T), dtype=np.float32)
    blob32_base[:, O_B1] = b1
    blob32_base[:, O_B2] = np.float32(b2)
    blob32_base[:, O_LNG:O_LNG + H] = ln_g[None, :]
    blob32_base[:, O_LNB:O_LNB + H] = ln_b[None, :]

    in_maps = []
    for c in range(NCORES):
        sl = slice(c * SPC, (c + 1) * SPC)
        blob32 = blob32_base.copy()
        blob32[0:C, O_CTS:O_CTS + SPC] = ctscale[sl].T
        in_maps.append({
            "xb": np.ascontiguousarray(xb[sl]),
            "chtp": np.ascontiguousarray(chtp[:, sl, :]),
            "blob16": blob16,
            "blob32": blob32,
        })
    return in_maps


def kernel(**inputs) -> tuple:
    in_maps = _marshal(inputs)
    from concourse import bass_utils
    nc = _get_nc()
    _CACHE["in_maps"] = in_maps
    res = bass_utils.run_bass_kernel_spmd(nc, in_maps, core_ids=list(range(NCORES)))

    tokens = np.empty((B, C, H), dtype=np.float32)
    Z = np.empty((B, C), dtype=np.float32)
    for c in range(NCORES):
        out = res.results[c]
        tokens[c * SPC:(c + 1) * SPC] = out["otok"]
        Z[c * SPC:(c + 1) * SPC] = out["oz"].T
    has_ch = Z > 0
    any_ch = has_ch.any(axis=0)
    tokens = np.where(any_ch[None, :, None], tokens, np.float32(0.0))
    return tokens, has_ch
